# revision 10
# baseline (speedup 1.0000x reference)
"""NT-Xent contrastive loss on 8 TRN2 NeuronCores.

Row-parallel over the 2B=8192 rows of z = concat(z_i, z_j).  Each core
receives the FULL z, rotated so its 1024-row block sits at rows 0:1024
(positive pairs then always sit at rows 4096:5120), and TRANSPOSED on the
host to [D, 2B] so the contraction dim is already on partitions - the SPMD
program is identical across cores with all-static offsets and no on-device
transposes.

Per core, pipelined over 8 column groups j of 1024 rows-of-z each:
  - DMA zT[128k:128k+128, 1024j:1024j+1024] fp32 (4 chunks of D=512)
  - squares on DVE (bf16), column norms^2 via ones-vector matmuls on PE
    (partition-dim reduce into PSUM [1,512])
  - inv-norm a = exp(-0.5*ln(n2)) on ScalarE (one ACT table set total)
  - broadcast a to all partitions via rank-1 matmul (ones[1,128].T @ a)
  - zn[k][j] = zT*a -> bf16 (DVE, column-normalized transposed z)
  - main: sim block = zn[:, 0:1024-block].T @ zn via bf16 matmuls,
    fused Exp(2x)+row-accumulate on ScalarE straight out of PSUM
  - positive-pair / self dots read from the sim PSUM diag stripes
    (identity-mask multiply-accumulate on DVE) before exp consumes them
  - loss rows = ln(rowsum - exp(2*self)) - 2*pos -> [1024] out
Host computes loss = mean(rows) over the 8x1024 gathered rows.
"""

import os
import sys

for _p in ("/opt/trn_rl_repo", "/opt/pypackages"):
    if os.path.isdir(_p) and _p not in sys.path:
        sys.path.append(_p)

import numpy as np

B = 4096
D = 512
N2 = 2 * B                  # 8192 rows total
NCORES = 8
RPC = N2 // NCORES          # 1024 rows per core
TAU_INV = 2.0               # 1 / temperature (temperature = 0.5)

NJ = 8                      # column groups of 1024
JW = N2 // NJ               # 1024
KC = D // 128               # 4 contraction chunks

_NC_CACHE = {}


def _build_nc():
    from contextlib import ExitStack

    import concourse.bacc as bacc
    import concourse.mybir as mybir
    import concourse.tile as tile
    from concourse.bass import ts
    from concourse.masks import make_identity

    f32 = mybir.dt.float32
    bf16 = mybir.dt.bfloat16
    fp8 = mybir.dt.float8e4
    PM = mybir.MatmulPerfMode
    AF = mybir.ActivationFunctionType
    ALU = mybir.AluOpType

    nc = bacc.Bacc("TRN2", target_bir_lowering=False, debug=False,
                   num_devices=NCORES)

    # Steer Ln AND Exp to the one table set containing both, so the
    # Ln/Exp alternation in the norm path doesn't reload ACT tables 21x.
    # Set list order/length is preserved (set ids stay valid); sets other
    # than natural_log_exp_and_others just stop advertising exp/ln to the
    # placement pass.  The real on-device tables are unchanged.
    def _act_loads_one_set():
        import bass_rust
        from concourse.hw_specs import get_activation_tables
        if not any(isinstance(i, mybir.InstActivation)
                   for b in nc.main_func.blocks for i in b.instructions):
            return
        tables = []
        for name, funcs in get_activation_tables(nc.m.arch).items():
            if name != "natural_log_exp_and_others":
                funcs = funcs - {mybir.ActivationFunctionType.Exp,
                                 mybir.ActivationFunctionType.Ln}
            tables.append((name, funcs))
        bass_rust.insert_act_table_loads(nc, tables)

    nc.insert_act_table_loads = _act_loads_one_set
    zt_dram = nc.dram_tensor("zt", [D, N2], bf16, kind="ExternalInput").ap()
    out_dram = nc.dram_tensor("out", [RPC], f32, kind="ExternalOutput").ap()

    with ExitStack() as ctx:
        tc = ctx.enter_context(tile.TileContext(nc))
        const = ctx.enter_context(tc.tile_pool(name="const", bufs=1))
        pzr = ctx.enter_context(tc.tile_pool(name="pzr", bufs=16))
        psq = ctx.enter_context(tc.tile_pool(name="psq", bufs=10))
        plog = ctx.enter_context(tc.tile_pool(name="plog", bufs=2))
        pej = ctx.enter_context(tc.tile_pool(name="pej", bufs=6))
        pdj = ctx.enter_context(tc.tile_pool(name="pdj", bufs=2))
        pps = ctx.enter_context(tc.tile_pool(name="pps", bufs=2, space="PSUM"))
        pbc = ctx.enter_context(tc.tile_pool(name="pbc", bufs=1, space="PSUM"))
        pn2 = ctx.enter_context(tc.tile_pool(name="pn2", bufs=1, space="PSUM"))
        keep = ctx.enter_context(tc.tile_pool(name="keep", bufs=1))

        ident = const.tile([128, 128], f32, name="ident", tag="ident")
        make_identity(nc, ident[:])
        ones_col2 = const.tile([128, 2, 16], fp8, name="ones_col2",
                               tag="ones_col2")
        nc.vector.memset(ones_col2[:], 1.0)
        ones_row = const.tile([1, 128], bf16, name="ones_row", tag="ones_row")
        nc.vector.memset(ones_row[:], 1.0)

        # persistent normalized transposed z, fp8 DoubleRow pair layout:
        # znT2[(kk, j)][p, s, c] = zn[D = 256*kk + 128*s + p, col c]
        znT2 = {(kk, j): keep.tile([128, 2, JW], fp8, name=f"zn_{kk}_{j}",
                                   tag=f"zn_{kk}_{j}")
                for kk in range(2) for j in range(NJ)}
        denp = keep.tile([128, 64], f32, name="denp", tag="denp")
        pos = keep.tile([128, 8], f32, name="pos", tag="pos")
        dself = keep.tile([128, 8], f32, name="dself", tag="dself")
        eself = keep.tile([128, 8], f32, name="eself", tag="eself")
        den8 = keep.tile([128, 8], f32, name="den8", tag="den8")
        den8b = keep.tile([128, 8], f32, name="den8b", tag="den8b")
        lden = keep.tile([128, 8], f32, name="lden", tag="lden")
        lossr = keep.tile([128, 8], f32, name="lossr", tag="lossr")

        def prep(j):
            """Load, norm, scale column group j into zn[(k, j)]."""
            ztr = {}
            sq = {}
            for k in range(KC):
                ztr[k] = pzr.tile([128, JW], bf16, name=f"ztr_{k}_{j}",
                                  tag="ztr")
                nc.sync.dma_start(
                    out=ztr[k][:],
                    in_=zt_dram[k * 128:(k + 1) * 128, j * JW:(j + 1) * JW])
            for kk in range(2):
                sq[kk] = psq.tile([128, 2, JW], fp8, name=f"sq_{kk}_{j}",
                                  tag="sq")
            for k in range(KC):
                nc.vector.tensor_mul(sq[k // 2][:, k % 2, :], ztr[k][:],
                                     ztr[k][:])
            acol = keep.tile([1, JW], bf16, name=f"acol_{j}", tag=f"acol_{j}")
            n2p = pn2.tile([1, JW], f32, name=f"n2p_{j}", tag="n2p")
            for half in range(2):
                for kk in range(2):
                    nc.tensor.matmul(n2p[:, ts(half, 512)],
                                     lhsT=ones_col2[:, :, 0:1],
                                     rhs=sq[kk][:, :, ts(half, 512)],
                                     start=(kk == 0), stop=(kk == 1),
                                     perf_mode=PM.DoubleRow)
            lnb = plog.tile([1, JW], f32, name=f"lnb_{j}", tag="lnb")
            nc.scalar.activation(out=lnb[:], in_=n2p[:], func=AF.Ln)
            nc.scalar.activation(out=acol[:], in_=lnb[:],
                                 func=AF.Exp, scale=-0.5)
            abc = pbc.tile([128, JW], f32, name=f"abc_{j}", tag="abc")
            for half in range(2):
                nc.tensor.matmul(abc[:, ts(half, 512)], lhsT=ones_row[:],
                                 rhs=acol[:, ts(half, 512)],
                                 start=True, stop=True)
            for k in range(KC):
                nc.vector.tensor_mul(znT2[(k // 2, j)][:, k % 2, :],
                                     ztr[k][:], abc[:])

        def main(ng):
            """Sim block matmuls + fused exp/rowsum for column group ng."""
            for m in range(8):
                ps = pps.tile([128, JW], f32, name=f"ps_{ng}_{m}", tag="ps")
                for kk in range(2):
                    for nn in range(2):
                        nc.tensor.matmul(
                            ps[:, ts(nn, 512)],
                            lhsT=znT2[(kk, 0)][:, :, ts(m, 128)],
                            rhs=znT2[(kk, ng)][:, :, ts(nn, 512)],
                            start=(kk == 0), stop=(kk == 1),
                            perf_mode=PM.DoubleRow)
                if ng == 0 or ng == 4:
                    # diag stripes: self-dots (ng=0) / positive pairs (ng=4)
                    tgt = dself if ng == 0 else pos
                    dj = pdj.tile([128, 128], f32, name=f"dj_{ng}_{m}",
                                  tag="dj")
                    nc.vector.scalar_tensor_tensor(
                        out=dj[:], in0=ps[:, ts(m, 128)], scalar=1.0,
                        in1=ident[:], op0=ALU.mult, op1=ALU.mult,
                        accum_out=tgt[:, m:m + 1])
                ej = pej.tile([128, JW], bf16, name=f"ej_{ng}_{m}", tag="ej")
                nc.scalar.activation(out=ej[:], in_=ps[:], func=AF.Exp,
                                     scale=TAU_INV,
                                     accum_out=denp[:, m * 8 + ng:
                                                    m * 8 + ng + 1])

        prep(0)
        prep(1)
        prep(2)
        for ng in range(NJ):
            main(ng)
            if ng + 3 < NJ:
                prep(ng + 3)

        # ---------- final combine ----------
        nc.scalar.activation(out=eself[:], in_=dself[:], func=AF.Exp,
                             scale=TAU_INV)
        nc.vector.tensor_reduce(
            out=den8[:], in_=denp.rearrange("p (m x) -> p m x", x=8),
            axis=mybir.AxisListType.X, op=ALU.add)
        nc.vector.tensor_sub(den8b[:], den8[:], eself[:])
        nc.scalar.activation(out=lden[:], in_=den8b[:], func=AF.Ln)
        nc.vector.scalar_tensor_tensor(
            out=lossr[:], in0=pos[:], scalar=-TAU_INV, in1=lden[:],
            op0=ALU.mult, op1=ALU.add)
        nc.sync.dma_start(out=out_dram.rearrange("(m p) -> p m", p=128),
                          in_=lossr[:])

    nc.compile()
    return nc


def _get_nc():
    if "nc" not in _NC_CACHE:
        _NC_CACHE["nc"] = _build_nc()
    return _NC_CACHE["nc"]


def _in_maps(z):
    import ml_dtypes
    zbf = z.astype(ml_dtypes.bfloat16)
    return [{"zt": np.ascontiguousarray(np.roll(zbf, -RPC * c, axis=0).T)}
            for c in range(NCORES)]


def kernel(z_i: np.ndarray, z_j: np.ndarray) -> np.ndarray:
    from concourse.bass_interp import get_hw_module
    from concourse.bass_utils import run_bass_kernel_spmd

    z = np.concatenate([np.asarray(z_i, np.float32),
                        np.asarray(z_j, np.float32)], axis=0)
    nc = _get_nc()
    old_m = nc.m
    nc.m = get_hw_module(nc.m)
    try:
        res = run_bass_kernel_spmd(nc, _in_maps(z),
                                   core_ids=list(range(NCORES)))
    finally:
        nc.m = old_m

    # loss = -mean(log(pos/den)) = mean(log(den) - 2*pos) = mean(rows)
    rows = np.concatenate([res.results[c]["out"] for c in range(NCORES)])
    return np.float32(np.mean(rows.astype(np.float64)))

